# revision 1
# baseline (speedup 1.0000x reference)
"""Causal MHA (shared q_linear) Bass kernel for 8 TRN2 NeuronCores.

Sharding: core c handles batch b=c//2, head-group g=c%2 (8 of 16 heads,
columns 512g:512g+512 of the shared projection).  Each core computes a
partial output (its head-group's contribution through Wo); the host sums
the two partials per batch and adds bo.

Compute layout (per core, S=2048 tokens, D=1024, 8 heads of hd=64):
  xT  = transpose(x) via PE               [1024, 2048]  (fp32, exact)
  qT/kT = Wq_g^T @ xT (+bq)               [512, 2048]   (fp32r matmuls)
  v   = x @ Wq_g (+bq), stored [tok, head, 65] with a fused ones column
  scoresT[k,q] = kh @ qh^T (per head, K=64, two heads packed in PE rows)
  exp on ACT with scale=1/8, additive -1e10 causal mask on PSUM
  attnT[hd+1, q] = [vh|1]^T @ expT  accumulated over k in PSUM
     row 64 = sum(exp) -> reciprocal -> rank-1 PE broadcast -> normalize
  out = attnT^T @ Wo_g  (partial, host adds the two head-groups + bo)
"""

import sys

sys.path.insert(0, "/opt/trn_rl_repo")

import numpy as np
import concourse.bass as bass  # noqa: F401
import concourse.tile as tile
from concourse import bacc, mybir
from concourse.bass_utils import run_bass_kernel_spmd

F32 = mybir.dt.float32
F32R = mybir.dt.float32r
BF16 = mybir.dt.bfloat16
AF = mybir.ActivationFunctionType

S = 2048          # tokens
D = 1024          # model dim
DL = 512          # local (per-core) projection columns = 8 heads * 64
HD = 64           # head dim
NHL = 8           # local heads
TB = 4            # token blocks of 512
JD = 8            # Din blocks of 128
NEG = -1.0e10


def build(repeat: int = 1, mode: str = "full", variant: str = "v4"):
    nc = bacc.Bacc("TRN2", target_bir_lowering=False, debug=False)
    xdt = BF16 if variant == "v7" else F32
    x_aps = {
        n: nc.dram_tensor(n, [S, D], xdt, kind="ExternalInput").ap()
        for n in ("x_q", "x_k", "x_v")
    }
    wq_ap = nc.dram_tensor("wq", [D, DL], F32, kind="ExternalInput").ap()
    bq_ap = nc.dram_tensor("bq", [DL], F32, kind="ExternalInput").ap()
    wo_ap = nc.dram_tensor("wo", [DL, D], F32, kind="ExternalInput").ap()
    tri_ap = nc.dram_tensor("tri", [128, 128], F32, kind="ExternalInput").ap()
    tri01_ap = nc.dram_tensor("tri01", [128, 128], F32, kind="ExternalInput").ap()
    id_ap = nc.dram_tensor("ident", [128, 128], F32, kind="ExternalInput").ap()
    out_ap = nc.dram_tensor("out", [S, D], F32, kind="ExternalOutput").ap()

    with tile.TileContext(nc) as tc:
        with tc.tile_pool(name="const", bufs=1) as const, \
             tc.tile_pool(name="persist", bufs=1) as persist, \
                          tc.tile_pool(name="xn", bufs=3) as xnp, \
             tc.tile_pool(name="xt", bufs=1) as xtp, \
             tc.tile_pool(name="qt", bufs=4) as qtp, \
             tc.tile_pool(name="exp", bufs=(2 if variant == "v9" else 3)) as ep, \
             tc.tile_pool(name="at", bufs=2) as atp, \
             tc.tile_pool(name="norm", bufs=1) as normp, \
             tc.tile_pool(name="ob", bufs=2) as obp, \
             tc.tile_pool(name="psS", bufs=(2 if variant == "v8" else 3), space="PSUM") as psS, \
             tc.tile_pool(name="psAcc", bufs=(3 if variant == "v8" else 2), space="PSUM") as psAcc:

            # ---- constants ----
            ident = const.tile([128, 128], F32)
            nc.sync.dma_start(ident[:], id_ap[:])
            tri = const.tile([128, 128], F32)
            nc.sync.dma_start(tri[:], tri_ap[:])
            tri01 = const.tile([128, 128], F32)
            nc.sync.dma_start(tri01[:], tri01_ap[:])
            bq_sb = const.tile([128, 4], F32)
            nc.sync.dma_start(bq_sb[:], bq_ap.rearrange("(t p) -> p t", p=128))
            bq_row = const.tile([1, DL], F32)
            nc.sync.dma_start(bq_row[:], bq_ap.rearrange("(a n) -> a n", a=1))
            bq_row_r = const.tile([1, DL], F32R)
            nc.vector.tensor_copy(bq_row_r[:], bq_row[:])
            ones_f = const.tile([128, 128], F32)
            nc.vector.memset(ones_f[:], 1.0)
            ones_r = const.tile([128, 128], F32R)
            nc.vector.tensor_copy(ones_r[:], ones_f[:])
            if variant == "v7":
                ones_b = const.tile([128, 128], BF16)
                nc.vector.tensor_copy(ones_b[:], ones_f[:])
                bq_row_b = const.tile([1, DL], BF16)
                nc.vector.tensor_copy(bq_row_b[:], bq_row[:])

            # ---- weights (cast to fp32r once) ----
            wq_r = persist.tile([128, JD, DL], BF16 if variant == "v7" else F32R)
            for j in range(JD):
                st = obp.tile([128, D], F32, tag="ob")
                nc.sync.dma_start(st[:, 0:DL], wq_ap[j * 128:(j + 1) * 128, :])
                nc.vector.tensor_copy(wq_r[:, j, :], st[:, 0:DL])
            wo_r = persist.tile([128, 4, D], F32R)
            for kt in range(4):
                st = obp.tile([128, D], F32, tag="ob")
                nc.sync.dma_start(st[:], wo_ap[kt * 128:(kt + 1) * 128, :])
                nc.vector.tensor_copy(wo_r[:, kt, :], st[:])

            # persistent per-token-block tensors
            kT = [persist.tile([128, 4, 512], F32R, name=f"kT{i}", tag=f"kT{i}") for i in range(TB)]
            vv = [persist.tile([128, 4, NHL, HD + 1], F32R, name=f"vv{i}", tag=f"vv{i}") for i in range(TB)]

            q_tiles = [None] * TB

            def _phase1_transpose(x_ap, xT, tb):
                for sub in range(4):
                    r0 = tb * 512 + sub * 128
                    # two half-tiles so transposes of D-cols 0:512 start as
                    # soon as the first 256KB lands (whole-tile dep otherwise
                    # stalls PE ~2.7us at every input boundary)
                    xh = []
                    for half in range(2):
                        xn = xnp.tile([128, DL], F32, tag=f"xn{half}")
                        nc.sync.dma_start(
                            xn[:], x_ap[r0:r0 + 128, half * DL:(half + 1) * DL]
                        )
                        xh.append(xn)
                    if variant not in ("v5", "v6"):
                        for jg in range(2):
                            pt = psS.tile([128, 512], F32, tag="sc")
                            for ji in range(4):
                                j = jg * 4 + ji
                                nc.tensor.transpose(
                                    pt[:, ji * 128:(ji + 1) * 128],
                                    xh[jg][:, ji * 128:(ji + 1) * 128],
                                    ident[:],
                                )
                            dst = xT[:, jg * 4:(jg + 1) * 4,
                                     sub * 128:(sub + 1) * 128]
                            srcv = pt[:].rearrange("p (j t) -> p j t", j=4)
                            if jg == 0 or variant == "v3":
                                nc.vector.tensor_copy(dst, srcv)
                            else:
                                nc.scalar.activation(dst, srcv, AF.Identity)
                    else:
                        pt = psS.tile([128, 2, 512], F32, tag="sc")
                        for j in range(JD):
                            nc.tensor.transpose(
                                pt[:, j // 4, (j % 4) * 128:(j % 4 + 1) * 128],
                                xn[:, j * 128:(j + 1) * 128],
                                ident[:],
                            )
                        dst = xT[:, :, sub * 128:(sub + 1) * 128]
                        srcv = pt[:].rearrange("p b (g t) -> p (b g) t", g=4)
                        if sub % 2 == 0:
                            nc.vector.tensor_copy(dst, srcv)
                        else:
                            nc.scalar.activation(dst, srcv, AF.Identity)

            def phase1(tb, rep):
                """transpose + project q,k,v for token block tb (512 tokens)."""
                for name in ("x_k", "x_v", "x_q"):
                    x_ap = x_aps[name]
                    if variant == "v7":
                        xT = xtp.tile([128, JD, 512], BF16, tag="xt")
                        for j in range(JD):
                            nc.scalar.dma_start(
                                out=xT[:, j, :],
                                in_=x_ap[tb * 512:(tb + 1) * 512,
                                         j * 128:(j + 1) * 128],
                                transpose=True,
                            )
                    else:
                        xT = xtp.tile([128, JD, 512], F32R, tag="xt")
                        _phase1_transpose(x_ap, xT, tb)
                    if name == "x_v":
                        vt = vv[tb]
                        for sub in range(4):
                            pv = psS.tile([128, 512], F32, tag="sc")
                            for j in range(JD):
                                nc.tensor.matmul(
                                    pv[:],
                                    xT[:, j, sub * 128:(sub + 1) * 128],
                                    wq_r[:, j, :],
                                    start=(j == 0),
                                    stop=False,
                                )
                            nc.tensor.matmul(
                                pv[:],
                                (ones_b if variant == "v7" else ones_r)[0:1, 0:128],
                                (bq_row_b if variant == "v7" else bq_row_r)[:],
                                start=False,
                                stop=True,
                            )
                            nc.vector.tensor_copy(
                                vt[:, sub, :, 0:HD],
                                pv[:].rearrange("p (h d) -> p h d", h=NHL),
                            )
                        nc.vector.tensor_copy(
                            vt[:, :, :, HD],
                            ones_f[:, 0:32].rearrange("p (s h) -> p s h", s=4),
                        )
                    else:
                        if name == "x_q":
                            dest = qtp.tile([128, 4, 512], F32R, tag="qt")
                            q_tiles[tb] = dest
                        else:
                            dest = kT[tb]
                        for dt_ in range(4):
                            py = psS.tile([128, 512], F32, tag="sc")
                            for j in range(JD):
                                nc.tensor.matmul(
                                    py[:],
                                    wq_r[:, j, dt_ * 128:(dt_ + 1) * 128],
                                    xT[:, j, :],
                                    start=(j == 0),
                                    stop=(j == JD - 1),
                                )
                            nc.scalar.activation(
                                dest[:, dt_, :],
                                py[:],
                                AF.Identity,
                                bias=bq_sb[:, dt_:dt_ + 1],
                            )

            def attention(Q, rep):
                """attention + Wo for query block Q (512 tokens)."""
                attnT = [atp.tile([128, 512], F32R, tag=f"at{i}", name=f"attnT{i}")
                         for i in range(4)]
                qtile = q_tiles[Q]
                nj = 4 * (Q + 1)
                for hp in range(4):
                    acc0 = psAcc.tile([128, 512], F32, tag="acc")
                    acc1 = psAcc.tile([128, 512], F32, tag="acc")

                    def emit_scores(j):
                        """scoresT pair + mask + exp for k-tile j; returns exp tile."""
                        tbj, sub = j // 4, j % 4
                        qoff = max(0, j * 128 - Q * 512)
                        ps = psS.tile([128, 2, 512], F32, tag="sc", name=f"ps{j}")
                        for hi, base in ((0, 0), (1, 64)):
                            nc.tensor.matmul(
                                ps[:, hi, qoff:],
                                kT[tbj][base:base + 64, hp,
                                        sub * 128:(sub + 1) * 128],
                                qtile[base:base + 64, hp, qoff:],
                                start=True,
                                stop=True,
                            )
                        diag = j * 128 >= Q * 512
                        if diag and variant != "v11":
                            for hi in range(2):
                                nc.vector.tensor_add(
                                    ps[:, hi, qoff:qoff + 128],
                                    ps[:, hi, qoff:qoff + 128],
                                    tri[:],
                                )
                        et = ep.tile([128, 2, 512], F32R, tag="exp", name=f"et{j}")
                        nc.scalar.activation(
                            et[:, :, qoff:], ps[:, :, qoff:], AF.Exp, scale=0.125
                        )
                        if diag and variant == "v11":
                            # zero masked entries after exp, off the PE->ACT chain
                            for hi in range(2):
                                nc.vector.tensor_mul(
                                    et[:, hi, qoff:qoff + 128],
                                    et[:, hi, qoff:qoff + 128],
                                    tri01[:],
                                )
                        return et

                    def emit_attn(j, et):
                        tbj, sub = j // 4, j % 4
                        qoff = max(0, j * 128 - Q * 512)
                        for hi, acc in ((0, acc0), (1, acc1)):
                            nc.tensor.matmul(
                                acc[0:65, qoff:],
                                vv[tbj][:, sub, hp * 2 + hi, :],
                                et[:, hi, qoff:],
                                start=(j == 0),
                                stop=(j == nj - 1),
                            )

                    # software pipeline: scores/exp run up to two k-tiles
                    # ahead of the accumulating attn matmuls so the in-order
                    # PE stream never head-blocks on the ACT exp.
                    depth = {"v3": 1, "v6": 3}.get(variant, 2)
                    ets = [emit_scores(j) for j in range(min(depth, nj))]
                    for j in range(depth, nj):
                        ets.append(emit_scores(j))
                        emit_attn(j - depth, ets[j - depth])
                    for j in range(max(0, nj - depth), nj):
                        emit_attn(j, ets[j])
                    if variant in ("v9",):
                        accs_sb = []
                        for hi, acc in ((0, acc0), (1, acc1)):
                            asb = normp.tile([128, 512], F32, tag=f"asb{hi}")
                            nc.vector.tensor_copy(asb[0:65, :], acc[0:65, :])
                            accs_sb.append(asb)
                        for hi, asb in ((0, accs_sb[0]), (1, accs_sb[1])):
                            sr = normp.tile([1, 512], F32, tag="sr")
                            nc.vector.tensor_copy(sr[0:1, :], asb[64:65, :])
                            bb = normp.tile([64, 512], F32, tag="bb")
                            nc.gpsimd.partition_broadcast(bb[:], sr[0:1, :])
                            rb = normp.tile([64, 512], F32, tag="rb")
                            nc.vector.reciprocal(rb[:], bb[:])
                            nc.vector.tensor_mul(
                                attnT[hp][hi * 64:(hi + 1) * 64, :],
                                asb[0:64, :],
                                rb[:],
                            )
                        continue_norm = False
                    else:
                        continue_norm = True
                    for hi, acc in (((0, acc0), (1, acc1)) if continue_norm else ()):
                        if variant == "v3":
                            sr = normp.tile([128, 512], F32R, tag="srr")
                            nc.vector.tensor_copy(sr[64:65, :], acc[64:65, :])
                            pb = psS.tile([128, 512], F32, tag="sc")
                            nc.tensor.matmul(
                                pb[0:64, :], ones_r[64:65, 0:64], sr[64:65, :],
                                start=True, stop=True,
                            )
                            rb = normp.tile([64, 512], F32, tag="rb")
                            nc.vector.reciprocal(rb[:], pb[0:64, :])
                        else:
                            # sum row -> DMA partition-broadcast -> wide
                            # reciprocal -> normalize (no PE/ACT involvement)
                            sr = normp.tile([1, 512], F32, tag="sr")
                            nc.vector.tensor_copy(sr[0:1, :], acc[64:65, :])
                            bb = normp.tile([64, 512], F32, tag="bb")
                            nc.gpsimd.partition_broadcast(bb[:], sr[0:1, :])
                            rb = normp.tile([64, 512], F32, tag="rb")
                            nc.vector.reciprocal(rb[:], bb[:])
                        nc.vector.tensor_mul(
                            attnT[hp][hi * 64:(hi + 1) * 64, :],
                            acc[0:64, :],
                            rb[:],
                        )
                # Wo projection for this token block
                for st_ in range(4):
                    ob = obp.tile([128, D], F32, tag="ob")
                    for nh in range(2):
                        po = psS.tile([128, 512], F32, tag="sc")
                        for kt in range(4):
                            nc.tensor.matmul(
                                po[:],
                                attnT[kt][:, st_ * 128:(st_ + 1) * 128],
                                wo_r[:, kt, nh * 512:(nh + 1) * 512],
                                start=(kt == 0),
                                stop=(kt == 3),
                            )
                        nc.vector.tensor_copy(ob[:, nh * 512:(nh + 1) * 512], po[:])
                    r0 = Q * 512 + st_ * 128
                    nc.sync.dma_start(out_ap[r0:r0 + 128, :], ob[:])

            if mode == "full":
                for rep in range(repeat):
                    if variant == "v10":
                        phase1(0, rep)
                        phase1(1, rep)
                        attention(0, rep)
                        phase1(2, rep)
                        attention(1, rep)
                        phase1(3, rep)
                        attention(2, rep)
                        attention(3, rep)
                    else:
                        for tb in range(TB):
                            phase1(tb, rep)
                        for Q in range(TB):
                            if Q == 0 and variant == "v12":
                                with tc.high_priority():
                                    attention(Q, rep)
                            else:
                                attention(Q, rep)
            elif mode == "p1":
                for rep in range(repeat):
                    for tb in range(TB):
                        phase1(tb, rep)
                for Q in range(TB):
                    attention(Q, 0)
            elif mode == "attn":
                for tb in range(TB):
                    phase1(tb, 0)
                for rep in range(repeat):
                    for Q in range(TB):
                        attention(Q, rep)

    nc.compile()
    return nc


_BUILD_CACHE = {}


def _get(repeat=1, mode="full", variant="v4"):
    key = (repeat, mode, variant)
    if key not in _BUILD_CACHE:
        _BUILD_CACHE[key] = build(repeat, mode, variant)
    return _BUILD_CACHE[key]


def make_in_maps(q, k, v, Wq, bq, Wo, bo, variant="v4"):
    import ml_dtypes
    xdt = ml_dtypes.bfloat16 if variant == "v7" else np.float32
    tri = np.where(
        np.arange(128)[:, None] <= np.arange(128)[None, :], 0.0, NEG
    ).astype(np.float32)
    tri01 = (tri == 0.0).astype(np.float32)
    ident = np.eye(128, dtype=np.float32)
    in_maps = []
    for c in range(8):
        b, g = c // 2, c % 2
        sl = slice(g * DL, (g + 1) * DL)
        in_maps.append({
            "x_q": np.ascontiguousarray(q[b]).astype(xdt),
            "x_k": np.ascontiguousarray(k[b]).astype(xdt),
            "x_v": np.ascontiguousarray(v[b]).astype(xdt),
            "wq": np.ascontiguousarray(Wq[:, sl]),
            "bq": np.ascontiguousarray(bq[sl]),
            "wo": np.ascontiguousarray(Wo[sl, :]),
            "tri": tri,
            "tri01": tri01,
            "ident": ident,
        })
    return in_maps


DEFAULT_VARIANT = "v4"


def kernel(q, k, v, Wq, bq, Wo, bo):
    q, k, v, Wq, bq, Wo, bo = (
        np.asarray(a, dtype=np.float32) for a in (q, k, v, Wq, bq, Wo, bo)
    )
    nc = _get(1, "full", DEFAULT_VARIANT)
    in_maps = make_in_maps(q, k, v, Wq, bq, Wo, bo, DEFAULT_VARIANT)
    res = run_bass_kernel_spmd(nc, in_maps, list(range(8)))
    B = q.shape[0]
    out = np.empty((B, S, D), dtype=np.float32)
    for b in range(B):
        out[b] = res.results[2 * b]["out"] + res.results[2 * b + 1]["out"] + bo
    return out



# revision 9
# speedup vs baseline: 1.7657x; 1.7657x over previous
"""Causal MHA (shared q_linear) Bass kernel for 8 TRN2 NeuronCores.

Sharding: core c handles batch b=c//2, head-group g=c%2 (8 of 16 heads,
columns 512g:512g+512 of the shared projection).  Each core computes a
partial output (its head-group's contribution through Wo); the host sums
the two partials per batch and adds bo.

Design (v15):
  - x is pre-transposed on the HOST (xT [D, S]): zero PE transposes.
  - bf16 staging for x, Wq, Wo, v, exp(scores) and attnT (PE matmuls run
    1 cycle/row either way; DMA bytes halve).  q/k kept fp32r so QK^T
    keeps ~fp32 accuracy.
  - causal mask applied on the PE: a 128-row bf16 matmul (trineg^T @ I)
    accumulates -1e10 into the masked triangle of the diagonal PSUM
    tile, so no vector op sits on the scores->exp->AV chain.
  - ACT (scalar engine) does ONLY the exp (the global ACT budget,
    ~147us, is below PE's ~245us).  Bias-adds / PSUM drains are split
    across DVE and Pool.
  - one software pipeline over (head-pair, k-tile): scores+exp run
    `depth` tiles ahead of the accumulating AV matmuls.
  - ACT-free PE work (Wo of the previous Q block, projection groups of
    later token blocks) is interleaved INTO the attention stream as
    "filler" tasks, placed at head-pair boundaries (absorbing the acc
    PSUM handoff) and spread to cover exp backlog.  Wo(Q) runs inside
    attention(Q+1); Wo(3) is the tail.
  - weights/x arrive in 256KB chunks, wq interleaved with the first x
    tensor, so the first matmul starts ~1.5us after kernel entry.

Per-core compute (S=2048, D=1024, 8 local heads of hd=64):
  qT/kT[dt]  = Wq_dt^T @ xT + bq    [128, 4, 512] fp32r  (dl on partitions)
  v          = xT^T @ Wq + bq       [128, 4, 8, 65] bf16, ones column fused
  scoresT    = kh @ qh^T per head   PSUM [128, 2, 512] fp32 (+ mask matmul)
  et         = exp(scoresT/8)       bf16
  attnT      = [vh|1]^T @ et        PSUM accum over k; row 64 = sum(exp)
  normalize  : drain acc->SBUF, sum row -> Pool broadcast -> DVE
               reciprocal -> multiply into attnT bf16
  out        = attnT^T @ Wo         fp32 partial, host adds head-groups + bo
"""

import sys

sys.path.insert(0, "/opt/trn_rl_repo")

import collections

import numpy as np
import concourse.bass as bass  # noqa: F401
import concourse.tile as tile
from concourse import bacc, mybir
from concourse.bass_utils import run_bass_kernel_spmd

F32 = mybir.dt.float32
F32R = mybir.dt.float32r
BF16 = mybir.dt.bfloat16
AF = mybir.ActivationFunctionType

S = 2048          # tokens
D = 1024          # model dim
DL = 512          # local (per-core) projection columns = 8 heads * 64
HD = 64           # head dim
NHL = 8           # local heads
TB = 4            # token blocks of 512
JD = 8            # Din blocks of 128
NEG = -1.0e10


def build(repeat: int = 1, mode: str = "full", variant: str = "v15"):
    depth = 2
    nc = bacc.Bacc("TRN2", target_bir_lowering=False, debug=False)
    x_aps = {
        n: nc.dram_tensor(n, [D, S], BF16, kind="ExternalInput").ap()
        for n in ("xt_q", "xt_k", "xt_v")
    }
    wq_ap = nc.dram_tensor("wq", [D, DL], BF16, kind="ExternalInput").ap()
    bq_ap = nc.dram_tensor("bq", [DL], F32, kind="ExternalInput").ap()
    wo_ap = nc.dram_tensor("wo", [DL, D], BF16, kind="ExternalInput").ap()
    trineg_t_ap = nc.dram_tensor(
        "trineg_t", [128, 128], BF16, kind="ExternalInput"
    ).ap()
    ident_ap = nc.dram_tensor("ident_b", [128, 128], BF16, kind="ExternalInput").ap()
    out_ap = nc.dram_tensor("out", [S, D], F32, kind="ExternalOutput").ap()

    with tile.TileContext(nc) as tc:
        with tc.tile_pool(name="const", bufs=1) as const, \
             tc.tile_pool(name="persist", bufs=1) as persist, \
             tc.tile_pool(name="xt", bufs=10) as xtp, \
             tc.tile_pool(name="exp", bufs=5) as ep, \
             tc.tile_pool(name="at", bufs=2) as atp, \
             tc.tile_pool(name="norm", bufs=2) as normp, \
             tc.tile_pool(name="ob", bufs=2) as obp, \
             tc.tile_pool(name="psS", bufs=2, space="PSUM") as psS, \
             tc.tile_pool(name="psA", bufs=2, space="PSUM") as psA, \
             tc.tile_pool(name="psP", bufs=2, space="PSUM") as psP:

            # ---- weights: wq in 4 chunks, interleaved with the first x
            # tensor's chunks so the first projection matmul starts early
            wqs = [persist.tile([128, 2, DL], BF16, name=f"wq{jj}", tag=f"wq{jj}")
                   for jj in range(4)]
            wq_r4 = wq_ap.rearrange("(jj j p) d -> jj p j d", jj=4, p=128)
            xk0_tiles = [xtp.tile([128, 2, 512], BF16, tag="xt", name=f"xk0_{jj}")
                         for jj in range(4)]
            xk0_r4 = x_aps["xt_k"].rearrange(
                "(jj j p) (b t) -> jj p j b t", jj=4, p=128, t=512
            )
            for jj in range(4):
                nc.sync.dma_start(wqs[jj][:], wq_r4[jj])
                nc.sync.dma_start(xk0_tiles[jj][:], xk0_r4[jj, :, :, 0, :])
            trineg = const.tile([128, 128], BF16)
            nc.sync.dma_start(trineg[:], trineg_t_ap[:])
            ident_b = const.tile([128, 128], BF16)
            nc.sync.dma_start(ident_b[:], ident_ap[:])
            bq_sb = const.tile([128, 4], F32)
            nc.sync.dma_start(bq_sb[:], bq_ap.rearrange("(t p) -> p t", p=128))
            bq_row = const.tile([1, DL], F32)
            nc.sync.dma_start(bq_row[:], bq_ap.rearrange("(a n) -> a n", a=1))
            bq_bc = const.tile([128, DL], F32)
            nc.gpsimd.partition_broadcast(bq_bc[:], bq_row[0:1, :])

            wo_r = persist.tile([128, 4, D], BF16)

            # persistent per-token-block tensors
            kT = [persist.tile([128, 4, 512], F32R, name=f"kT{i}", tag=f"kT{i}")
                  for i in range(TB)]
            qT = [persist.tile([128, 4, 512], F32R, name=f"qT{i}", tag=f"qT{i}")
                  for i in range(TB)]
            vv = [persist.tile([128, 4, NHL, HD + 1], BF16, name=f"vv{i}",
                               tag=f"vv{i}") for i in range(TB)]
            for i in range(TB):
                nc.vector.memset(vv[i][:, :, :, HD], 1.0)

            # ---------- phase-1 task factories ----------
            # One "input" = one of xt_{k,v,q} for one token block: a dma task
            # (4 chunk DMAs) plus 4 matmul groups (one PSUM tile each).
            def p1_input(tb, name, premade=None):
                holder = {}

                def dma():
                    if premade is not None:
                        holder["x"] = premade
                        return
                    xts = [xtp.tile([128, 2, 512], BF16, tag="xt",
                                    name=f"x_{name}{tb}_{jj}") for jj in range(4)]
                    x_r4 = x_aps[name].rearrange(
                        "(jj j p) (b t) -> jj p j b t", jj=4, p=128, t=512
                    )
                    for jj in range(4):
                        nc.sync.dma_start(xts[jj][:], x_r4[jj, :, :, tb, :])
                    holder["x"] = xts

                def grp(i, eng):
                    xts = holder["x"]
                    if name == "xt_v":
                        pv = psP.tile([128, 512], F32, tag="pp",
                                      name=f"pv{tb}_{i}")
                        for j in range(JD):
                            nc.tensor.matmul(
                                pv[:],
                                xts[j // 2][:, j % 2, i * 128:(i + 1) * 128],
                                wqs[j // 2][:, j % 2, :],
                                start=(j == 0),
                                stop=(j == JD - 1),
                            )
                        nc.vector.tensor_add(
                            vv[tb][:, i, :, 0:HD],
                            pv[:].rearrange("p (h d) -> p h d", h=NHL),
                            bq_bc[:].rearrange("p (h d) -> p h d", h=NHL),
                        )
                    else:
                        dest = kT[tb] if name == "xt_k" else qT[tb]
                        py = psP.tile([128, 512], F32, tag="pp",
                                      name=f"py{tb}_{i}")
                        for j in range(JD):
                            nc.tensor.matmul(
                                py[:],
                                wqs[j // 2][:, j % 2, i * 128:(i + 1) * 128],
                                xts[j // 2][:, j % 2, :],
                                start=(j == 0),
                                stop=(j == JD - 1),
                            )
                        nc.vector.tensor_scalar_add(
                            dest[:, i, :], py[:], bq_sb[:, i:i + 1]
                        )

                groups = [(lambda i=i: grp(i, None)) for i in range(4)]
                return dma, groups

            # ---------- Wo task factory ----------
            def wo_groups(Q, attnTs):
                obs = {}

                def grp(st_, nh):
                    if nh == 0:
                        obs[st_] = obp.tile([128, D], F32, tag="ob",
                                            name=f"ob{Q}_{st_}")
                    po = psP.tile([128, 512], F32, tag="pp", name=f"po{Q}_{st_}{nh}")
                    for kt in range(4):
                        nc.tensor.matmul(
                            po[:],
                            attnTs[kt][:, st_ * 128:(st_ + 1) * 128],
                            wo_r[:, kt, nh * 512:(nh + 1) * 512],
                            start=(kt == 0),
                            stop=(kt == 3),
                        )
                    nc.vector.tensor_copy(
                        obs[st_][:, nh * 512:(nh + 1) * 512], po[:]
                    )
                    if nh == 1:
                        r0 = Q * 512 + st_ * 128
                        nc.sync.dma_start(out_ap[r0:r0 + 128, :], obs[st_][:])

                return [
                    (lambda st_=st_, nh=nh: grp(st_, nh))
                    for st_ in range(4) for nh in range(2)
                ]

            # ---------- attention ----------
            def attention(Q, fillers=()):
                """fillers: list of (closure, max_idx|None).  Returns the 4
                normalized attnT tiles (Wo is the caller's business)."""
                attnT = [atp.tile([128, 512], BF16, tag=f"at{i}",
                                  name=f"attnT{Q}_{i}") for i in range(4)]
                qtile = qT[Q]
                nj = 4 * (Q + 1)
                T = 4 * nj
                accs = {}

                # schedule fillers: constrained ones early (before their
                # deadline), then one per head-pair boundary (absorbs the acc
                # PSUM handoff), rest spread evenly.
                # candidate positions: head-pair boundaries (absorb the acc
                # PSUM handoff) + an even spread; assigned to fillers IN
                # ORDER (list order is execution order), clamped to each
                # filler's deadline.
                sched = collections.defaultdict(list)
                if fillers:
                    n = len(fillers)
                    positions = set(hp * nj + depth for hp in range(1, 4))
                    step = max(1, T // (n + 1))
                    p = step
                    while len(positions) < n:
                        positions.add(min(p, T - 1))
                        p += step
                        if p > 4 * T:
                            break
                    positions = sorted(positions)[:n]
                    while len(positions) < n:
                        positions.append(T - 1)
                    for (f, mx), pos in zip(fillers, positions):
                        sched[min(pos, mx) if mx is not None else pos].append(f)

                def emit_scores(hp, j):
                    tbj, sub = j // 4, j % 4
                    qoff = max(0, j * 128 - Q * 512)
                    diag = j * 128 >= Q * 512
                    ps = psS.tile([128, 2, 512], F32, tag="sc", name=f"ps{hp}_{j}")
                    for hi in range(2):
                        nc.tensor.matmul(
                            ps[:, hi, qoff:],
                            kT[tbj][64 * hi:64 * hi + 64, hp,
                                    sub * 128:(sub + 1) * 128],
                            qtile[64 * hi:64 * hi + 64, hp, qoff:],
                            start=True,
                            stop=not diag,
                        )
                        if diag:
                            # accumulate -1e10 into the masked triangle on
                            # the PE (128-row bf16 matmul, ~53ns)
                            nc.tensor.matmul(
                                ps[:, hi, qoff:qoff + 128],
                                trineg[:],
                                ident_b[:],
                                start=False,
                                stop=True,
                                skip_group_check=True,
                            )
                    et = ep.tile([128, 2, 512], BF16, tag="exp", name=f"et{hp}_{j}")
                    nc.scalar.activation(
                        et[:, :, qoff:], ps[:, :, qoff:], AF.Exp, scale=0.125
                    )
                    return et

                def emit_attn(hp, j, et):
                    tbj, sub = j // 4, j % 4
                    qoff = max(0, j * 128 - Q * 512)
                    if j == 0:
                        acc0 = psA.tile([128, 512], F32, tag="acc",
                                        name=f"acc0_{hp}")
                        acc1 = psA.tile([128, 512], F32, tag="acc",
                                        name=f"acc1_{hp}")
                        accs[hp] = (acc0, acc1)
                    for hi in range(2):
                        nc.tensor.matmul(
                            accs[hp][hi][0:65, qoff:],
                            vv[tbj][:, sub, hp * 2 + hi, :],
                            et[:, hi, qoff:],
                            start=(j == 0),
                            stop=(j == nj - 1),
                        )

                def normalize(hp):
                    # drain acc PSUM to SBUF first (frees both acc banks in
                    # ~600ns), then normalize entirely in SBUF
                    a0 = normp.tile([65, 512], F32, tag="asb0", name=f"a0_{hp}")
                    a1 = normp.tile([65, 512], F32, tag="asb1", name=f"a1_{hp}")
                    nc.vector.tensor_copy(a0[:], accs[hp][0][0:65, :])
                    nc.vector.tensor_copy(a1[:], accs[hp][1][0:65, :])
                    for hi, a in ((0, a0), (1, a1)):
                        sr = normp.tile([1, 512], F32, tag=f"sr{hi}",
                                        name=f"sr{hi}_{hp}")
                        nc.vector.tensor_copy(sr[0:1, :], a[64:65, :])
                        bb = normp.tile([64, 512], F32, tag=f"bb{hi}",
                                        name=f"bb{hi}_{hp}")
                        nc.gpsimd.partition_broadcast(bb[:], sr[0:1, :])
                        rb = normp.tile([64, 512], F32, tag=f"rb{hi}",
                                        name=f"rb{hi}_{hp}")
                        nc.vector.reciprocal(rb[:], bb[:])
                        nc.gpsimd.tensor_mul(
                            attnT[hp][64 * hi:64 * hi + 64, :],
                            a[0:64, :],
                            rb[:],
                        )

                tasks = [(hp, j) for hp in range(4) for j in range(nj)]
                ets = {}
                for idx, (hp, j) in enumerate(tasks):
                    ets[(hp, j)] = emit_scores(hp, j)
                    for f in sched.get(idx, ()):
                        f()
                    if idx >= depth:
                        phl, jl = tasks[idx - depth]
                        emit_attn(phl, jl, ets.pop((phl, jl)))
                        if jl == nj - 1:
                            normalize(phl)
                for idx in range(len(tasks) - depth, len(tasks)):
                    phl, jl = tasks[idx]
                    emit_attn(phl, jl, ets.pop((phl, jl)))
                    if jl == nj - 1:
                        normalize(phl)
                for idx in sorted(sched):
                    if idx >= T:
                        for f in sched[idx]:
                            f()
                return attnT

            # ---------- orchestration ----------
            def emit_full(rep):
                # tb0: k-input chunks were pre-DMA'd interleaved with wq
                d_k0, g_k0 = p1_input(0, "xt_k",
                                      premade=xk0_tiles if rep == 0 else None)
                d_v0, g_v0 = p1_input(0, "xt_v")
                d_q0, g_q0 = p1_input(0, "xt_q")
                d_k0(); d_v0(); d_q0()
                for g in g_k0 + g_v0 + g_q0:
                    g()
                if rep == 0:
                    nc.sync.dma_start(
                        wo_r[:], wo_ap.rearrange("(k p) d -> p k d", p=128)
                    )
                # tb1 as a block
                p1g1 = []
                for name in ("xt_k", "xt_v", "xt_q"):
                    d, gs = p1_input(1, name)
                    d()
                    p1g1 += gs
                for g in p1g1:
                    g()
                # attention(0) with the first 2 groups of tb2-k as fillers
                d_k2, g_k2 = p1_input(2, "xt_k")
                d_k2()
                at0 = attention(0, fillers=[(g, None) for g in g_k2[:2]])
                # rest of tb2
                for g in g_k2[2:]:
                    g()
                p1g2 = []
                for name in ("xt_v", "xt_q"):
                    d, gs = p1_input(2, name)
                    d()
                    p1g2 += gs
                for g in p1g2:
                    g()
                # attention(1) with Wo(0) as fillers
                at1 = attention(1, fillers=[(g, None) for g in wo_groups(0, at0)])
                # attention(2): Wo(1) + tb3 q-projection as fillers
                d_q3, g_q3 = p1_input(3, "xt_q")
                d_q3()
                at2 = attention(
                    2,
                    fillers=[(g, None) for g in wo_groups(1, at1)]
                    + [(g, None) for g in g_q3],
                )
                # attention(3): Wo(2) unconstrained + tb3 k/v constrained
                # before their first use (k-dt needed at task idx 16*dt+12,
                # v-sub at idx ~12+sub)
                d_k3, g_k3 = p1_input(3, "xt_k")
                d_v3, g_v3 = p1_input(3, "xt_v")
                d_k3(); d_v3()
                fill3 = [(g_k3[0], 3), (g_v3[0], 5), (g_v3[1], 7),
                         (g_v3[2], 9), (g_v3[3], 11), (g_k3[1], 24),
                         (g_k3[2], 40), (g_k3[3], 56)]
                fill3 += [(g, None) for g in wo_groups(2, at2)]
                at3 = attention(3, fillers=fill3)
                # tail: Wo(3)
                for g in wo_groups(3, at3):
                    g()

            for rep in range(repeat):
                if mode == "full":
                    emit_full(rep)
                else:
                    # simple un-interleaved paths for microbenchmarks
                    if rep == 0:
                        nc.sync.dma_start(
                            wo_r[:], wo_ap.rearrange("(k p) d -> p k d", p=128)
                        )
                    if mode == "p1":
                        for tb in range(TB):
                            for name in ("xt_k", "xt_v", "xt_q"):
                                d, gs = p1_input(
                                    tb, name,
                                    premade=xk0_tiles
                                    if (rep == 0 and tb == 0 and name == "xt_k")
                                    else None,
                                )
                                d()
                                for g in gs:
                                    g()
                    elif mode == "attn":
                        if rep == 0:
                            for tb in range(TB):
                                for name in ("xt_k", "xt_v", "xt_q"):
                                    d, gs = p1_input(
                                        tb, name,
                                        premade=xk0_tiles
                                        if (tb == 0 and name == "xt_k")
                                        else None,
                                    )
                                    d()
                                    for g in gs:
                                        g()
                        for Q in range(TB):
                            atq = attention(Q)
                            for g in wo_groups(Q, atq):
                                g()

    nc.compile()
    return nc


_BUILD_CACHE = {}


def _get(repeat=1, mode="full", variant="v15"):
    key = (repeat, mode, variant)
    if key not in _BUILD_CACHE:
        _BUILD_CACHE[key] = build(repeat, mode, variant)
    return _BUILD_CACHE[key]


def make_in_maps(q, k, v, Wq, bq, Wo, bo, variant="v15"):
    import ml_dtypes
    bf16 = ml_dtypes.bfloat16
    # trineg[p, c] = 0 if p <= c else NEG  (k row p masked for q col c < p);
    # the kernel accumulates trineg into PSUM via trineg_t^T @ I = trineg.
    trineg = np.where(
        np.arange(128)[:, None] <= np.arange(128)[None, :], 0.0, NEG
    ).astype(np.float32)
    trineg_t = np.ascontiguousarray(trineg.T).astype(bf16)
    ident = np.eye(128, dtype=np.float32).astype(bf16)
    in_maps = []
    for c in range(8):
        b, g = c // 2, c % 2
        sl = slice(g * DL, (g + 1) * DL)
        in_maps.append({
            "xt_q": np.ascontiguousarray(q[b].T).astype(bf16),
            "xt_k": np.ascontiguousarray(k[b].T).astype(bf16),
            "xt_v": np.ascontiguousarray(v[b].T).astype(bf16),
            "wq": np.ascontiguousarray(Wq[:, sl]).astype(bf16),
            "bq": np.ascontiguousarray(bq[sl]),
            "wo": np.ascontiguousarray(Wo[sl, :]).astype(bf16),
            "trineg_t": trineg_t,
            "ident_b": ident,
        })
    return in_maps


DEFAULT_VARIANT = "v15"


def kernel(q, k, v, Wq, bq, Wo, bo):
    q, k, v, Wq, bq, Wo, bo = (
        np.asarray(a, dtype=np.float32) for a in (q, k, v, Wq, bq, Wo, bo)
    )
    nc = _get(1, "full", DEFAULT_VARIANT)
    in_maps = make_in_maps(q, k, v, Wq, bq, Wo, bo, DEFAULT_VARIANT)
    res = run_bass_kernel_spmd(nc, in_maps, list(range(8)))
    B = q.shape[0]
    out = np.empty((B, S, D), dtype=np.float32)
    for b in range(B):
        out[b] = res.results[2 * b]["out"] + res.results[2 * b + 1]["out"] + bo
    return out
